# revision 37
# baseline (speedup 1.0000x reference)
"""Distributed causal multi-head attention for one TRN2 chip (8 NeuronCores).

Problem: x[4, 2048, 1024], 16 heads x 64 dim, causal attention + in/out proj.

Sharding: core = (batch b, head-group hg): b = core // 2, hg = core % 2.
Each core computes QKV for its batch's 8 heads, full causal attention, and
the output projection restricted to its 512 y-channels (a partial sum).
The host combines each pair of partials (bf16 partials, f32 sum) -- no
cross-core communication is needed on device.

Layout (activations bf16 in SBUF, f32 PSUM accumulation):
 - x is passed transposed and t-chunk-major (xt [4*1024, 512]) so the
   contraction dim (channels) is on SBUF partitions and every DMA tile is a
   fully contiguous 128 KB block.  wq/wk are passed colc-major so each
   128-col generation group's 8 weight tiles are one contiguous 256 KB
   stream, letting the first gen group start ~2 us after launch.
 - Attention scores are computed transposed, ST[j, i] = (K q^T)^T, so the
   AV matmul needs no transpose of the softmax matrix.  The two heads of a
   head-pair live on partition halves 0:64 / 64:128; bass auto-derives
   tile_position from base_partition, so each score pair runs CONCURRENTLY
   in the two row-halves of the PE array (K=64 row tiling).
 - exp without max subtraction (scores are O(1) by construction); masked
   diagonal blocks are zeroed after exp with a multiplicative mask; the
   softmax denominator comes free from a ones-column interleaved into V
   (65th output row of the AV matmul).
 - Normalization multiplies by reciprocal sums: PSUM denominator row ->
   SBUF copy, DVE reciprocal_approx_fast, GpSimd partition_broadcast, DVE
   multiply.  The two multiplies are DEFERRED a few weave slots so they
   don't head-of-line block the DVE queue behind the GpSimd broadcast.
 - The whole kernel is ONE flat software-pipelined block stream:
   S(b) pair -> [paced filler] -> AV(b-1) pair, with chunk boundaries
   handled inline (the exp stream never pauses).  Filler = QKV generation
   and projection matmuls at SINGLE-MATMUL granularity, paced by a
   cycle-count model so the PE absorbs exactly the per-block ACT slack.
   A dependency ledger (need()) force-drains filler groups right before
   the attention stream reads their outputs.
 - PSUM: 2x [128,1024] score supertiles (pS) + 4x [128,512] (pO: chunk
   AV accumulators A/B + up to 2 in-flight filler accumulators).
 - A burst of zero matmuls at t=0 pre-warms the PE HAM clock gate while
   the first DMAs land.  Output is written bf16 (halves the out DMA).
"""

import numpy as np
import ml_dtypes

B, T, C = 4, 2048, 1024
H, D = 16, 64
HPC = 8            # heads per core
NCORES = 8
CH = HPC * D       # channels per core (512)
VW = HPC * 65      # v width: per head [v 64 | ones 1] (tight)

_BF16 = ml_dtypes.bfloat16
_F8 = ml_dtypes.float8_e4m3fn
USE_FP8_QK = False  # q/k generation via fp8e4m3 DoubleRow matmuls (2x K)
W8SCALE = 32.0      # host premultiply so weights clear the fp8 subnormals

_nc_cache = {}
LAST_RESULT = [None]  # BassKernelResults of the most recent run (for profiling)


def _fix_multi_waits(nc):
    """This toolchain's walrus accepts at most ONE sync-wait per
    instruction; Tile's final drain batches several.  Split extra waits
    into single-wait NoOps placed immediately before on the same engine."""
    import bass_rust
    from concourse import mybir

    ctr = 0
    for f in nc.m.functions:
        for bb in f.blocks:
            out, changed = [], False
            for inst in bb.instructions:
                si = inst.sync_info
                if si is not None and len(si.on_wait) > 1:
                    waits = list(si.on_wait)
                    for w in waits[:-1]:
                        ctr += 1
                        nop = mybir.InstNoOp(name=f"xwait_{ctr}", ins=[], outs=[])
                        nop.engine = inst.engine
                        nop.sync_info = bass_rust.SyncInfo(on_wait=[w], on_update=[])
                        out.append(nop)
                    inst.sync_info = bass_rust.SyncInfo(
                        on_wait=[waits[-1]], on_update=list(si.on_update))
                    changed = True
                out.append(inst)
            if changed:
                bb.instructions = out


def _enable_ldw_opt():
    # measured ~10us faster and numerically identical on this toolchain
    try:
        from concourse.compiler_utils import get_compiler_flags, \
            set_compiler_flags
        flags = [f.replace("--enable-ldw-opt=false", "--enable-ldw-opt=true")
                 for f in get_compiler_flags()]
        set_compiler_flags(flags)
    except Exception:
        pass


class _Filler:
    """Ordered queue of single-matmul filler thunks with group ids.

    emit(n_ns) pays out ~n_ns of PE work; need(gid) force-drains through
    the end of group gid.  Deferred (zero-cost) thunks fire after a given
    number of paid-out items."""

    def __init__(self):
        self.items = []          # list of (gid, cost_ns, fn)
        self.pos = 0
        self.done = set()
        self.deferred = []       # list of [countdown, fn]
        self.spent_total = 0.0

    def add_group(self, gid, thunks):
        # thunks: list of (cost_ns, fn)
        for c, fn in thunks:
            self.items.append((gid, c, fn))

    def _tick_deferred(self):
        fire = [d for d in self.deferred if d[0] <= 0]
        self.deferred = [d for d in self.deferred if d[0] > 0]
        for _, fn in fire:
            fn()
        for d in self.deferred:
            d[0] -= 1

    def defer(self, fn, slots):
        self.deferred.append([slots, fn])

    def flush_deferred(self):
        for _, fn in self.deferred:
            fn()
        self.deferred = []

    def _emit_one(self):
        gid, c, fn = self.items[self.pos]
        fn()
        self.pos += 1
        self.spent_total += c
        if self.pos >= len(self.items) or self.items[self.pos][0] != gid:
            self.done.add(gid)
        self._tick_deferred()
        return c

    def need(self, gid):
        spent = 0
        while gid not in self.done:
            assert self.pos < len(self.items), f"filler underflow for {gid}"
            spent += self._emit_one()
        return spent

    def emit(self, budget_ns):
        spent = 0
        while self.pos < len(self.items) and spent < budget_ns:
            spent += self._emit_one()
        return spent

    def drain(self):
        while self.pos < len(self.items):
            self._emit_one()
        # fire any stragglers
        for _, fn in self.deferred:
            fn()
        self.deferred = []


def build_nc(fix_waits=True, use_bias=False):
    import concourse.tile as tile
    from concourse import bacc, mybir
    from contextlib import ExitStack

    _enable_ldw_opt()

    BF = mybir.dt.bfloat16
    F32 = mybir.dt.float32
    EXP = mybir.ActivationFunctionType.Exp
    ESCALE = 0.125 / (W8SCALE * W8SCALE) if (USE_FP8_QK and not use_bias) \
        else 0.125

    nc = bacc.Bacc()
    # all inputs are row-paired on the host: channel chunks (2a, 2a+1) sit
    # side by side in one 128-partition tile, so every DMA is ONE ~256 KB
    # contiguous descriptor (dma_start issue costs ~650 ns of engine queue
    # time each -- descriptor count, not bytes, dominated the old startup)
    xt_d = nc.declare_dram_parameter("xt", [4 * 512, 1024], BF, isOutput=False)
    wq_d = nc.declare_dram_parameter("wq", [512, 1024], BF, isOutput=False)
    wk_d = nc.declare_dram_parameter("wk", [512, 1024], BF, isOutput=False)
    wv_d = nc.declare_dram_parameter("wv", [512, 1024], BF, isOutput=False)
    wp_d = nc.declare_dram_parameter("wp", [256, 2 * C], BF, isOutput=False)
    mk_d = nc.declare_dram_parameter("msk", [128, 4 * 512], BF, isOutput=False)
    fp8_qk = USE_FP8_QK and not use_bias
    if fp8_qk:
        F8 = mybir.dt.float8e4
        DR = mybir.MatmulPerfMode.DoubleRow
        x8_d = nc.declare_dram_parameter("x8", [1024, 2048], F8,
                                         isOutput=False)
        wq8_d = nc.declare_dram_parameter("wq8", [512, 1024], F8,
                                          isOutput=False)
        wk8_d = nc.declare_dram_parameter("wk8", [512, 1024], F8,
                                          isOutput=False)
    if use_bias:
        bq_d = nc.declare_dram_parameter("bq", [CH, 1], F32, isOutput=False)
        bk_d = nc.declare_dram_parameter("bk", [CH, 1], F32, isOutput=False)
        bv_d = nc.declare_dram_parameter("bv", [1, CH], F32, isOutput=False)
        bp_d = nc.declare_dram_parameter("bp", [1, C], F32, isOutput=False)
    out_d = nc.declare_dram_parameter("out", [T, C], BF, isOutput=True)

    with tile.TileContext(nc) as tc, ExitStack() as ctx:
        persist = ctx.enter_context(tc.tile_pool(name="persist", bufs=1))

        # persistent SBUF tensors
        qt = [persist.tile([128, T], BF, tag=f"qt{i}", name=f"qt{i}") for i in range(4)]
        kt = [persist.tile([128, T], BF, tag=f"kt{i}", name=f"kt{i}") for i in range(4)]
        vt = [persist.tile([128, VW], BF, tag=f"vt{i}", name=f"vt{i}") for i in range(16)]
        yt = [persist.tile([128, T], BF, tag=f"yt{i}", name=f"yt{i}") for i in range(4)]
        msk = persist.tile([128, 4 * 512], BF, tag="msk", name="msk")
        wup = persist.tile([128, 512], BF, tag="wup", name="wup")

        with tc.tile_pool(name="pS", bufs=2, space="PSUM") as pS, \
             tc.tile_pool(name="pO", bufs=4, space="PSUM") as pO, \
             tc.tile_pool(name="wqk", bufs=1) as wqkp, \
             tc.tile_pool(name="x8", bufs=4) as x8p, \
             tc.tile_pool(name="wv", bufs=1) as wvp, \
             tc.tile_pool(name="wp", bufs=1) as wpp, \
             tc.tile_pool(name="xt", bufs=8) as xtp, \
             tc.tile_pool(name="outst", bufs=8) as outp, \
             tc.tile_pool(name="exp", bufs=6) as expp, \
             tc.tile_pool(name="rn", bufs=4) as rnp:

            # ---- PE warm-up: zero matmuls while the first DMAs land ----
            nc.vector.memset(wup[:], 0.0)
            wps = pS.tile([128, 512], F32, tag="S", name="Swu")

            def wu_mm():
                nc.tensor.matmul(wps[:], wup[:, 0:128], wup[:],
                                 start=True, stop=True)

            # ---- DMAs: one 256 KB descriptor per tile-pair, issue spread
            # over the GpSimd (weights) and Sync (x, out) queues so the
            # ~650 ns/descriptor issue cost parallelizes.
            xts_all = {}

            def load_xts(tcx):
                xts_all[tcx] = []
                for a in range(4):
                    t = xtp.tile([128, 1024], BF, tag="xt", name="xt")
                    r0 = tcx * 512 + a * 128
                    nc.sync.dma_start(t[:], xt_d[r0:r0 + 128, :])
                    xts_all[tcx].append(t)

            wq_sb, wk_sb, wv_sb = [], [], []
            x8_all = {}

            def load_x8(tcx):
                x8_all[tcx] = []
                for sp in range(2):
                    t = x8p.tile([128, 2048], F8, tag="x8", name="x8")
                    r0 = tcx * 256 + sp * 128
                    nc.sync.dma_start(t[:], x8_d[r0:r0 + 128, :])
                    x8_all[tcx].append(t)

            if fp8_qk:
                wq8_sb, wk8_sb = [], []
                for colc in range(4):
                    t = wqkp.tile([128, 1024], F8, tag=f"wq8_{colc}",
                                  name=f"wq8_{colc}")
                    nc.gpsimd.dma_start(t[:],
                                        wq8_d[colc * 128:(colc + 1) * 128, :])
                    wq8_sb.append(t)
                    t = wqkp.tile([128, 1024], F8, tag=f"wk8_{colc}",
                                  name=f"wk8_{colc}")
                    nc.gpsimd.dma_start(t[:],
                                        wk8_d[colc * 128:(colc + 1) * 128, :])
                    wk8_sb.append(t)
                load_x8(0)
                load_xts(0)
                load_x8(1)
            else:
                for a in range(4):
                    t = wqkp.tile([128, 1024], BF, tag=f"wq{a}", name=f"wq{a}")
                    nc.gpsimd.dma_start(t[:], wq_d[a * 128:(a + 1) * 128, :])
                    wq_sb.append(t)
                load_xts(0)
                for a in range(4):
                    t = wqkp.tile([128, 1024], BF, tag=f"wk{a}", name=f"wk{a}")
                    nc.gpsimd.dma_start(t[:], wk_d[a * 128:(a + 1) * 128, :])
                    wk_sb.append(t)
            nc.gpsimd.dma_start(msk[:], mk_d[:, :])
            for a in range(4):
                t = wvp.tile([128, 1024], BF, tag=f"wv{a}", name=f"wv{a}")
                nc.gpsimd.dma_start(t[:], wv_d[a * 128:(a + 1) * 128, :])
                wv_sb.append(t)
            load_xts(1)
            wp_sb = []
            for a in range(2):
                t = wpp.tile([128, 2 * C], BF, tag=f"wp{a}", name=f"wp{a}")
                nc.sync.dma_start(t[:], wp_d[a * 128:(a + 1) * 128, :])
                wp_sb.append(t)

            # warm the PE HAM clock gate while the first DMAs land
            for _ in range(12):
                wu_mm()

            # ones columns of V (tight layout: col 64 of each 65-wide head)
            for i in range(16):
                v3 = vt[i][:].rearrange("p (h c) -> p h c", h=8, c=65)
                nc.vector.memset(v3[:, :, 64:65], 1.0)

            def wq_sl(colc, ck):
                a, par = ck >> 1, ck & 1
                cs = slice(par * 512 + colc * 128, par * 512 + (colc + 1) * 128)
                return wq_sb[a][:, cs]

            def wk_sl(colc, ck):
                a, par = ck >> 1, ck & 1
                cs = slice(par * 512 + colc * 128, par * 512 + (colc + 1) * 128)
                return wk_sb[a][:, cs]

            def x_sl(tcx, ck):
                a, par = ck >> 1, ck & 1
                return xts_all[tcx][a][:, par * 512:(par + 1) * 512]

            def xst_sl(tcx, ck, tt):
                a, par = ck >> 1, ck & 1
                cs = slice(par * 512 + tt * 128, par * 512 + (tt + 1) * 128)
                return xts_all[tcx][a][:, cs]

            def wv_sl(ck):
                a, par = ck >> 1, ck & 1
                return wv_sb[a][:, par * 512:(par + 1) * 512]

            def wp_sl(ck, cc):
                a, par = ck >> 1, ck & 1
                cs = slice(par * C + cc * 512, par * C + (cc + 1) * 512)
                return wp_sb[a][:, cs]

            if fp8_qk:
                def qk_mm(ps, which, colc, tcx, a, start, stop):
                    w8 = (wq8_sb if which == "q" else wk8_sb)[colc]
                    lhsT = w8[:, a * 256:(a + 1) * 256].rearrange(
                        "p (i m) -> p i m", i=2, m=128)
                    sp, al = a >> 1, a & 1
                    rhs = x8_all[tcx][sp][:, al * 1024:(al + 1) * 1024
                                          ].rearrange("p (i n) -> p i n",
                                                      i=2, n=512)
                    nc.tensor.matmul(ps, lhsT, rhs, start=start, stop=stop,
                                     perf_mode=DR)
                N_QK = 4
            else:
                def qk_mm(ps, which, colc, tcx, ck, start, stop):
                    w_sl = wq_sl if which == "q" else wk_sl
                    nc.tensor.matmul(ps, w_sl(colc, ck), x_sl(tcx, ck),
                                     start=start, stop=stop)
                N_QK = 8

            if use_bias:
                bq_sb = persist.tile([128, 4], F32, tag="bq", name="bq")
                bk_sb = persist.tile([128, 4], F32, tag="bk", name="bk")
                bv_row = persist.tile([1, CH], F32, tag="bvr", name="bvr")
                bp_row = persist.tile([1, C], F32, tag="bpr", name="bpr")
                bvb = persist.tile([128, CH], F32, tag="bvb", name="bvb")
                bpb = persist.tile([128, C], F32, tag="bpb", name="bpb")
                for colc in range(4):
                    nc.sync.dma_start(bq_sb[:, colc:colc + 1],
                                      bq_d[colc * 128:(colc + 1) * 128, :])
                    nc.sync.dma_start(bk_sb[:, colc:colc + 1],
                                      bk_d[colc * 128:(colc + 1) * 128, :])
                nc.sync.dma_start(bv_row[:], bv_d[:, :])
                nc.sync.dma_start(bp_row[:], bp_d[:, :])
                nc.gpsimd.partition_broadcast(bvb[:], bv_row[:], channels=128)
                nc.gpsimd.partition_broadcast(bpb[:], bp_row[:], channels=128)

            # ---- filler thunk builders (single-matmul granularity) ----
            MM = 221.0   # ns, one N=512 bf16 matmul issue-to-issue

            def palloc(nm):
                # Every pO allocation may reuse the ring slot of a chunk
                # accumulator whose deferred yt-multiplies haven't been
                # emitted yet -- flush them first so Tile sees the reads.
                fill.flush_deferred()
                return pO.tile([128, 512], F32, tag="O", name=nm)

            def qk_group(which, colc, tcx):
                """matmuls + copy producing qt/kt[colc][:, tcx-chunk]."""
                dst = qt if which == "q" else kt
                ts = slice(tcx * 512, (tcx + 1) * 512)
                cell = {}

                def mk(ck):
                    def f():
                        if ck == 0:
                            cell["ps"] = palloc("Sg")
                        qk_mm(cell["ps"][:], which, colc, tcx, ck,
                              ck == 0, ck == N_QK - 1)
                        if ck == N_QK - 1:
                            if use_bias:
                                bcol = bq_sb if which == "q" else bk_sb
                                nc.vector.tensor_scalar_add(
                                    dst[colc][:, ts], cell["ps"][:],
                                    bcol[:, colc:colc + 1])
                            else:
                                nc.vector.tensor_copy(dst[colc][:, ts],
                                                      cell["ps"][:])
                    return f
                return [(MM, mk(ck)) for ck in range(N_QK)]

            def v_group(tcx, tt):
                """8 matmuls + copy producing vt[tcx*4+tt]."""
                vti = vt[tcx * 4 + tt]
                cell = {}

                def mk(ck):
                    def f():
                        if ck == 0:
                            cell["ps"] = palloc("Vg")
                        nc.tensor.matmul(cell["ps"][:],
                                         xst_sl(tcx, ck, tt),
                                         wv_sl(ck),
                                         start=(ck == 0), stop=(ck == 7))
                        if ck == 7:
                            dst = vti[:].rearrange(
                                "p (h c) -> p h c", h=8, c=65)[:, :, 0:64]
                            src = cell["ps"][:].rearrange(
                                "p (h c) -> p h c", h=8, c=64)
                            if use_bias:
                                bsrc = bvb[:].rearrange(
                                    "p (h c) -> p h c", h=8, c=64)
                                nc.vector.tensor_add(dst, src, bsrc)
                            else:
                                nc.vector.tensor_copy(dst, src)
                    return f
                return [(MM, mk(ck)) for ck in range(8)]

            stage3 = {}  # t2 -> [128, 1024] f32 staged ck0-2 partial

            def proj3_partial(t2, cc):
                """ck=0..2 partial contraction for an i-chunk-3 output
                tile, staged to SBUF f32 (frees its PSUM slot right away);
                woven into the last attention chunk."""
                t2s = slice(t2 * 128, (t2 + 1) * 128)
                ccs = slice(cc * 512, (cc + 1) * 512)
                cell = {}

                def mk(ck):
                    def f():
                        if t2 not in stage3:
                            stage3[t2] = outp.tile([128, 1024], F32,
                                                   tag="st3", name="st3",
                                                   bufs=4)
                        if ck == 0:
                            cell["ps"] = palloc("Pp")
                        nc.tensor.matmul(cell["ps"][:], yt[ck][:, t2s],
                                         wp_sl(ck, cc),
                                         start=(ck == 0), stop=(ck == 2))
                        if ck == 2:
                            if use_bias:
                                nc.vector.tensor_add(stage3[t2][:, ccs],
                                                     cell["ps"][:],
                                                     bpb[:, ccs])
                            else:
                                nc.vector.tensor_copy(stage3[t2][:, ccs],
                                                      cell["ps"][:])
                    return f
                return [(MM, mk(ck)) for ck in range(3)]

            def proj_t2(t2):
                """8 matmuls (two 512-col halves), staged into one SBUF
                tile, one contiguous 256 KB output DMA."""
                t2s = slice(t2 * 128, (t2 + 1) * 128)
                cell = {}

                def mk(cc, ck):
                    ccs = slice(cc * 512, (cc + 1) * 512)

                    def f():
                        if cc == 0 and ck == 0:
                            cell["st"] = outp.tile([128, 1024], BF, tag="ost",
                                                   name="ost")
                        if ck == 0:
                            cell["ps"] = palloc("Sp")
                        nc.tensor.matmul(
                            cell["ps"][:], yt[ck][:, t2s], wp_sl(ck, cc),
                            start=(ck == 0), stop=(ck == 3))
                        if ck == 3:
                            if use_bias:
                                nc.vector.tensor_add(cell["st"][:, ccs],
                                                     cell["ps"][:], bpb[:, ccs])
                            else:
                                nc.vector.tensor_copy(cell["st"][:, ccs],
                                                      cell["ps"][:])
                            if cc == 1:
                                nc.sync.dma_start(out_d[t2s, :], cell["st"][:])
                    return f
                return [(MM, mk(cc, ck)) for cc in range(2) for ck in range(4)]

            # ---- build the filler queue in dependency-safe order ----
            fill = _Filler()
            for tt in range(4):
                fill.add_group(("v", 0, tt), v_group(0, tt))
            for colc in range(1, 4):
                fill.add_group(("k", colc, 0), qk_group("k", colc, 0))
                fill.add_group(("q", colc, 0), qk_group("q", colc, 0))
            for tcx in range(1, 4):
                def xload(tcx=tcx):
                    if tcx + 1 <= 3:
                        load_xts(tcx + 1)
                        if fp8_qk:
                            load_x8(tcx + 1)
                fill.add_group(("x", tcx), [(0.0, xload)])
                fill.add_group(("q", 0, tcx), qk_group("q", 0, tcx))
                fill.add_group(("k", 0, tcx), qk_group("k", 0, tcx))
                for tt in range(4):
                    fill.add_group(("v", tcx, tt), v_group(tcx, tt))
                for colc in range(1, 4):
                    fill.add_group(("q", colc, tcx), qk_group("q", colc, tcx))
                    fill.add_group(("k", colc, tcx), qk_group("k", colc, tcx))


            # ---- startup: the two gen groups the first chunk needs ----
            ps_q = palloc("q00")
            for ck in range(N_QK):
                qk_mm(ps_q[:], "q", 0, 0, ck, ck == 0, ck == N_QK - 1)
            if use_bias:
                nc.vector.tensor_scalar_add(qt[0][:, 0:512], ps_q[:],
                                            bq_sb[:, 0:1])
            else:
                nc.vector.tensor_copy(qt[0][:, 0:512], ps_q[:])
            ps_k = palloc("k00")
            for ck in range(N_QK):
                qk_mm(ps_k[:], "k", 0, 0, ck, ck == 0, ck == N_QK - 1)
            if use_bias:
                nc.vector.tensor_scalar_add(kt[0][:, 0:512], ps_k[:],
                                            bk_sb[:, 0:1])
            else:
                nc.vector.tensor_copy(kt[0][:, 0:512], ps_k[:])

            # ---- the flat attention block stream ----
            def normalize(hp, pic, opsA, opsB):
                """Phase 1 (copies/recips/broadcasts) runs now; the two DVE
                multiplies are deferred so they don't block the DVE queue
                while the GpSimd broadcast runs."""
                r = {}
                r["dnA"] = rnp.tile([1, 512], F32, tag="dn", name="dnA")
                nc.vector.tensor_copy(r["dnA"][:], opsA[D:D + 1, :])
                r["rfA"] = rnp.tile([1, 512], F32, tag="rf", name="rfA")
                nc.vector.reciprocal_approx_fast(r["rfA"][:], r["dnA"][:])
                r["rsA"] = rnp.tile([D, 512], F32, tag="Rs", name="RsA")
                nc.gpsimd.partition_broadcast(r["rsA"][:], r["rfA"][:],
                                              channels=D)
                r["dnB"] = rnp.tile([1, 512], F32, tag="dn", name="dnB")
                nc.vector.tensor_copy(r["dnB"][:], opsB[D:D + 1, :])
                r["rfB"] = rnp.tile([1, 512], F32, tag="rf", name="rfB")
                nc.vector.reciprocal_approx_fast(r["rfB"][:], r["dnB"][:])
                r["rsB"] = rnp.tile([D, 512], F32, tag="Rs", name="RsB")
                nc.gpsimd.partition_broadcast(r["rsB"][:], r["rfB"][:],
                                              channels=D)
                isl = slice(pic * 512, (pic + 1) * 512)

                def ph2(hp=hp, isl=isl, r=r, opsA=opsA, opsB=opsB):
                    nc.vector.tensor_mul(yt[hp][0:D, isl], opsA[0:D, :],
                                         r["rsA"][:])
                    nc.vector.tensor_mul(yt[hp][D:128, isl], opsB[0:D, :],
                                         r["rsB"][:])
                fill.defer(ph2, 4)

            blocks = []
            for ic in range(4):
                for hp in range(4):
                    jmax = 4 * (ic + 1)
                    for jt in range(jmax):
                        blocks.append((hp, ic, jt, jt == 0, jt == jmax - 1))

            # ---- global filler plan: late (ACT-bound) chunks get exactly
            # their ACT-over-PE deficit; the excess filler is front-loaded
            # into the early PE-bound chunks where emission order is free.
            def blk_costs(ic, jt):
                m = jt - 4 * ic
                c0 = 128 * m if m > 0 else 0
                a = (2 * (512 - c0) + 352) / 1.2
                p = (512 - c0) / 2.4 + 95.0 + 2 * (512 - c0) / 2.4 + 115.0
                return a, p

            # queue items now + proj(0..2) groups appended during the stream
            F_total = sum(c for _, c, _ in fill.items) + 12 * 8 * MM \
                + 24 * MM
            deficit = {}
            for ic in range(4):
                for hp in range(4):
                    a = p = 0.0
                    for jt in range(4 * (ic + 1)):
                        da, dp = blk_costs(ic, jt)
                        a += da
                        p += dp
                    deficit[(hp, ic)] = max(0.0, a - p - 300.0)
            extra = max(0.0, F_total - sum(deficit.values()))
            quota = dict(deficit)
            for ic in range(4):
                for hp in range(4):
                    if extra <= 0:
                        break
                    cap = 7000.0 if ic < 2 else 3000.0
                    add = min(extra, cap)
                    quota[(hp, ic)] += add
                    extra -= add
            # cumulative planned filler line per block index
            plan = []
            cum = 0.0
            for (hp, ic, jt, first, last) in blocks:
                cum += quota[(hp, ic)] / (4 * (ic + 1))
                plan.append(cum)

            pe_t = 0.0       # modeled PE completion time of emitted work
            act_t = 0.0      # modeled ACT completion time
            pend = None      # (hp, ic, jt, ex, c0, last, opsA, opsB, exp_end)
            cur_ops = None   # (opsA, opsB) of the chunk whose AVs are pending
            GUARD = 150.0
            bi = -1

            for (hp, ic, jt, first, last) in blocks:
                bi += 1
                # --- force-drain dependencies for this block's S pair ---
                tcx_j = jt // 4
                if first and (hp, ic) != (0, 0):
                    fill.need(("q", hp, ic))
                if (hp, tcx_j) != (0, 0):
                    fill.need(("k", hp, tcx_j))

                # --- emit S pair for this block ---
                jsl = slice(jt * 128, (jt + 1) * 128)
                m = jt - 4 * ic
                c0 = 128 * m if m > 0 else 0
                iv = slice(ic * 512 + c0, (ic + 1) * 512)
                sps = pS.tile([128, 1024], F32, tag="S", name="S")
                nc.tensor.matmul(sps[:, c0:512], kt[hp][0:D, jsl],
                                 qt[hp][0:D, iv], start=True, stop=True)
                nc.tensor.matmul(sps[:, 512 + c0:1024], kt[hp][D:128, jsl],
                                 qt[hp][D:128, iv], start=True, stop=True)
                s_cost = (512 - c0) / 2.4 + 95.0
                pe_t += s_cost

                # --- emit exp (+ mask for diagonal blocks) ---
                ex = expp.tile([128, 1024], BF, tag="ex", name="ex")
                ex3 = ex[:].rearrange("p (t c) -> p t c", t=2, c=512)
                sps3 = sps[:].rearrange("p (t c) -> p t c", t=2, c=512)
                if m < 0:
                    nc.scalar.activation(ex[:], sps[:], EXP, scale=ESCALE)
                else:
                    ms3 = msk[:, m * 512 + c0:m * 512 + c0 + 128
                              ].unsqueeze(1).broadcast_to([128, 2, 128])
                    nc.scalar.activation(ex3[:, :, c0:512],
                                         sps3[:, :, c0:512],
                                         EXP, scale=ESCALE)
                    nc.vector.tensor_mul(ex3[:, :, c0:c0 + 128],
                                         ex3[:, :, c0:c0 + 128], ms3)
                exp_start = max(act_t, pe_t + 60.0)
                act_t = exp_start + (2 * (512 - c0) + 352) / 1.2
                my_exp_end = act_t

                # --- pace filler until the pending AV's exp is done ---
                if pend is not None:
                    target = pend[8] + GUARD
                    if pe_t < target:
                        pe_t += fill.emit(target - pe_t)

                # --- emit the pending block's AV pair ---
                if pend is not None:
                    phs, pic_, pj, pex, pc0, plast, popsA, popsB, pexp = pend
                    v0 = 130 * phs
                    pe_t = max(pe_t, pexp + 80.0)
                    nc.tensor.matmul(popsA[0:65, pc0:512],
                                     vt[pj][:, v0:v0 + 65],
                                     pex[:, pc0:512],
                                     start=(pj == 0), stop=plast)
                    nc.tensor.matmul(popsB[0:65, pc0:512],
                                     vt[pj][:, v0 + 65:v0 + 130],
                                     pex[:, 512 + pc0:1024],
                                     start=(pj == 0), stop=plast)
                    pe_t += 2 * (512 - pc0) / 2.4 + 115.0
                    if plast:
                        normalize(phs, pic_, popsA, popsB)
                        if phs == 3 and pic_ < 3:
                            # all of i-chunk pic_ is normalized: its
                            # projection becomes available filler
                            for t2 in range(4 * pic_, 4 * pic_ + 4):
                                fill.add_group(("p", pic_, t2), proj_t2(t2))
                        if phs == 2 and pic_ == 3:
                            # hp 0..2 of i-chunk 3 are normalized: their
                            # projection partials weave into the last chunk
                            for t2 in range(12, 16):
                                for cc in range(2):
                                    fill.add_group(("p3", t2, cc),
                                                   proj3_partial(t2, cc))


                # --- planned-quota weaving (front-loads excess filler) ---
                if fill.spent_total < plan[bi]:
                    pe_t += fill.emit(plan[bi] - fill.spent_total)

                # --- queue this block as pending (alloc ops at jt==0) ---
                if first:
                    # deferred yt-multiplies of older chunks must be emitted
                    # before their ops PSUM slots can be reallocated
                    fill.flush_deferred()
                    opsA = pO.tile([128, 512], F32, tag="O", name="OA")
                    opsB = pO.tile([128, 512], F32, tag="O", name="OB")
                    cur_ops = (opsA, opsB)
                fill.need(("v", tcx_j, jt % 4))
                pend = (hp, ic, jt, ex, c0, last, cur_ops[0], cur_ops[1],
                        my_exp_end)

            # flush the final AV pair
            phs, pic_, pj, pex, pc0, plast, popsA, popsB, pexp = pend
            v0 = 130 * phs
            nc.tensor.matmul(popsA[0:65, pc0:512], vt[pj][:, v0:v0 + 65],
                             pex[:, pc0:512], start=(pj == 0), stop=True)
            nc.tensor.matmul(popsB[0:65, pc0:512],
                             vt[pj][:, v0 + 65:v0 + 130],
                             pex[:, 512 + pc0:1024],
                             start=(pj == 0), stop=True)
            normalize(phs, pic_, popsA, popsB)
            fill.drain()

            # ---- tail: ck=3 finisher of the i-chunk-3 projection ----
            for t2 in range(12, 16):
                t2s = slice(t2 * 128, (t2 + 1) * 128)
                ost = outp.tile([128, 1024], BF, tag="ost", name="ost")
                for cc in range(2):
                    ccs = slice(cc * 512, (cc + 1) * 512)
                    ps = palloc("Pt")
                    nc.tensor.matmul(ps[:], yt[3][:, t2s], wp_sl(3, cc),
                                     start=True, stop=True)
                    nc.vector.tensor_add(ost[:, ccs], stage3[t2][:, ccs],
                                         ps[:])
                nc.sync.dma_start(out_d[t2s, :], ost[:])

    nc.finalize()  # Bacc.compile(): ISA-subclass codegen, gpsimd library
    # loads, act-table loads, nop fusion -- must precede the wait splitting
    if fix_waits:
        _fix_multi_waits(nc)
    return nc


def _host_inputs(x, W_qkv, b_qkv, W_proj, b_proj, use_bias):
    x = np.asarray(x, np.float32)
    W_qkv = np.asarray(W_qkv, np.float32)
    b_qkv = np.asarray(b_qkv, np.float32)
    W_proj = np.asarray(W_proj, np.float32)
    b_proj = np.asarray(b_proj, np.float32)

    # causal masks for the 4 diagonal-overlap offsets: ST block [j 128, i 512]
    # at j0 - i0 = 128*m keeps (ii >= jj + 128*m)
    jj = np.arange(128)[:, None]
    ii = np.arange(512)[None, :]
    msk = np.concatenate(
        [(ii >= jj + 128 * m).astype(np.float32) for m in range(4)], axis=1)

    def pair_rows(M):
        # [R, C] -> [R/2, 2C]: row chunks (2a, 2a+1) side by side, so one
        # 128-partition SBUF tile loads as a single contiguous DMA
        R, Cc = M.shape
        return M.reshape(R // 256, 2, 128, Cc).transpose(0, 2, 1, 3).reshape(
            R // 2, 2 * Cc)

    in_maps = []
    for core in range(NCORES):
        b, hg = core >> 1, core & 1
        q0 = hg * CH
        xT = x[b].T  # [C, T]
        # t-chunk-major, row-paired
        xt = np.concatenate(
            [pair_rows(xT[:, tc * 512:(tc + 1) * 512]) for tc in range(4)],
            axis=0).astype(_BF16)
        wq = pair_rows(W_qkv[:, q0:q0 + CH]).astype(_BF16)
        wk = pair_rows(W_qkv[:, C + q0:C + q0 + CH]).astype(_BF16)
        wv = pair_rows(W_qkv[:, 2 * C + q0:2 * C + q0 + CH]).astype(_BF16)
        wp = pair_rows(W_proj[q0:q0 + CH, :]).astype(_BF16)
        im = {"xt": xt, "wq": wq, "wk": wk, "wv": wv, "wp": wp,
              "msk": msk.astype(_BF16)}
        if USE_FP8_QK and not use_bias:
            # x8[tcx, sp, p, a_loc, i, n] = x[256(2sp+a_loc)+128i+p, tcx*512+n]
            x8 = xT.reshape(2, 2, 2, 128, 4, 512).transpose(
                4, 0, 3, 1, 2, 5).reshape(1024, 2048)
            im["x8"] = x8.astype(_F8)

            def w8pack(w):
                # w8[colc][p, a*256+i*128+m] = W8SCALE * w[a*256+i*128+p,
                #                                          colc*128+m]
                wr = (W8SCALE * w).reshape(4, 2, 128, 4, 128)
                return wr.transpose(3, 2, 0, 1, 4).reshape(512, 1024).astype(
                    _F8)
            im["wq8"] = w8pack(W_qkv[:, q0:q0 + CH])
            im["wk8"] = w8pack(W_qkv[:, C + q0:C + q0 + CH])
        if use_bias:
            im["bq"] = b_qkv[q0:q0 + CH].reshape(CH, 1).astype(np.float32)
            im["bk"] = b_qkv[C + q0:C + q0 + CH].reshape(CH, 1).astype(
                np.float32)
            im["bv"] = b_qkv[2 * C + q0:2 * C + q0 + CH].reshape(1, CH).astype(
                np.float32)
            im["bp"] = (0.5 * b_proj).reshape(1, C).astype(np.float32)
        in_maps.append(im)
    return in_maps


def kernel(x, W_qkv, b_qkv, W_proj, b_proj):
    from concourse.bass_utils import run_bass_kernel_spmd

    use_bias = bool(np.any(np.asarray(b_qkv)) or np.any(np.asarray(b_proj)))
    if use_bias not in _nc_cache:
        _nc_cache[use_bias] = build_nc(use_bias=use_bias)
    nc = _nc_cache[use_bias]

    in_maps = _host_inputs(x, W_qkv, b_qkv, W_proj, b_proj, use_bias)
    res = run_bass_kernel_spmd(nc, in_maps, core_ids=list(range(NCORES)))
    LAST_RESULT[0] = res

    out = np.empty((B, T, C), np.float32)
    for b in range(B):
        out[b] = (res.results[2 * b]["out"].astype(np.float32)
                  + res.results[2 * b + 1]["out"].astype(np.float32))
    return out


# revision 38
# speedup vs baseline: 1.2137x; 1.2137x over previous
"""Distributed causal multi-head attention for one TRN2 chip (8 NeuronCores).

Problem: x[4, 2048, 1024], 16 heads x 64 dim, causal attention + in/out proj.

Sharding: core = (batch b, head-group hg): b = core // 2, hg = core % 2.
Each core computes QKV for its batch's 8 heads, full causal attention, and
the output projection restricted to its 512 y-channels (a partial sum).
The host combines each pair of partials (bf16 partials, f32 sum) -- no
cross-core communication is needed on device.

Layout (activations bf16 in SBUF, f32 PSUM accumulation):
 - x is passed transposed and t-chunk-major (xt [4*1024, 512]) so the
   contraction dim (channels) is on SBUF partitions and every DMA tile is a
   fully contiguous 128 KB block.  wq/wk are passed colc-major so each
   128-col generation group's 8 weight tiles are one contiguous 256 KB
   stream, letting the first gen group start ~2 us after launch.
 - Attention scores are computed transposed, ST[j, i] = (K q^T)^T, so the
   AV matmul needs no transpose of the softmax matrix.  The two heads of a
   head-pair live on partition halves 0:64 / 64:128; bass auto-derives
   tile_position from base_partition, so each score pair runs CONCURRENTLY
   in the two row-halves of the PE array (K=64 row tiling).
 - exp without max subtraction (scores are O(1) by construction); masked
   diagonal blocks are zeroed after exp with a multiplicative mask; the
   softmax denominator comes free from a ones-column interleaved into V
   (65th output row of the AV matmul).
 - Normalization multiplies by reciprocal sums: PSUM denominator row ->
   SBUF copy, DVE reciprocal_approx_fast, GpSimd partition_broadcast, DVE
   multiply.  The two multiplies are DEFERRED a few weave slots so they
   don't head-of-line block the DVE queue behind the GpSimd broadcast.
 - The whole kernel is ONE flat software-pipelined block stream:
   S(b) pair -> [paced filler] -> AV(b-1) pair, with chunk boundaries
   handled inline (the exp stream never pauses).  Filler = QKV generation
   and projection matmuls at SINGLE-MATMUL granularity, paced by a
   cycle-count model so the PE absorbs exactly the per-block ACT slack.
   A dependency ledger (need()) force-drains filler groups right before
   the attention stream reads their outputs.
 - PSUM: 2x [128,1024] score supertiles (pS) + 4x [128,512] (pO: chunk
   AV accumulators A/B + up to 2 in-flight filler accumulators).
 - A burst of zero matmuls at t=0 pre-warms the PE HAM clock gate while
   the first DMAs land.  Output is written bf16 (halves the out DMA).
"""

import numpy as np
import ml_dtypes

B, T, C = 4, 2048, 1024
H, D = 16, 64
HPC = 8            # heads per core
NCORES = 8
CH = HPC * D       # channels per core (512)
VW = HPC * 65      # v width: per head [v 64 | ones 1] (tight)

_BF16 = ml_dtypes.bfloat16
_F8 = ml_dtypes.float8_e4m3fn
USE_FP8_QK = False  # q/k generation via fp8e4m3 DoubleRow matmuls (2x K)
W8SCALE = 32.0      # host premultiply so weights clear the fp8 subnormals

_nc_cache = {}
LAST_RESULT = [None]  # BassKernelResults of the most recent run (for profiling)


def _fix_multi_waits(nc):
    """This toolchain's walrus accepts at most ONE sync-wait per
    instruction; Tile's final drain batches several.  Split extra waits
    into single-wait NoOps placed immediately before on the same engine."""
    import bass_rust
    from concourse import mybir

    ctr = 0
    for f in nc.m.functions:
        for bb in f.blocks:
            out, changed = [], False
            for inst in bb.instructions:
                si = inst.sync_info
                if si is not None and len(si.on_wait) > 1:
                    waits = list(si.on_wait)
                    for w in waits[:-1]:
                        ctr += 1
                        nop = mybir.InstNoOp(name=f"xwait_{ctr}", ins=[], outs=[])
                        nop.engine = inst.engine
                        nop.sync_info = bass_rust.SyncInfo(on_wait=[w], on_update=[])
                        out.append(nop)
                    inst.sync_info = bass_rust.SyncInfo(
                        on_wait=[waits[-1]], on_update=list(si.on_update))
                    changed = True
                out.append(inst)
            if changed:
                bb.instructions = out


def _enable_ldw_opt():
    # measured ~10us faster and numerically identical on this toolchain
    try:
        from concourse.compiler_utils import get_compiler_flags, \
            set_compiler_flags
        flags = [f.replace("--enable-ldw-opt=false", "--enable-ldw-opt=true")
                 for f in get_compiler_flags()]
        set_compiler_flags(flags)
    except Exception:
        pass


class _Filler:
    """Ordered queue of single-matmul filler thunks with group ids.

    emit(n_ns) pays out ~n_ns of PE work; need(gid) force-drains through
    the end of group gid.  Deferred (zero-cost) thunks fire after a given
    number of paid-out items."""

    def __init__(self):
        self.items = []          # list of (gid, cost_ns, fn)
        self.pos = 0
        self.done = set()
        self.deferred = []       # list of [countdown, fn]
        self.spent_total = 0.0

    def add_group(self, gid, thunks):
        # thunks: list of (cost_ns, fn)
        for c, fn in thunks:
            self.items.append((gid, c, fn))

    def _tick_deferred(self):
        fire = [d for d in self.deferred if d[0] <= 0]
        self.deferred = [d for d in self.deferred if d[0] > 0]
        for _, fn in fire:
            fn()
        for d in self.deferred:
            d[0] -= 1

    def defer(self, fn, slots):
        self.deferred.append([slots, fn])

    def flush_deferred(self):
        for _, fn in self.deferred:
            fn()
        self.deferred = []

    def _emit_one(self):
        gid, c, fn = self.items[self.pos]
        fn()
        self.pos += 1
        self.spent_total += c
        if self.pos >= len(self.items) or self.items[self.pos][0] != gid:
            self.done.add(gid)
        self._tick_deferred()
        return c

    def need(self, gid):
        spent = 0
        while gid not in self.done:
            assert self.pos < len(self.items), f"filler underflow for {gid}"
            spent += self._emit_one()
        return spent

    def emit(self, budget_ns):
        spent = 0
        while self.pos < len(self.items) and spent < budget_ns:
            spent += self._emit_one()
        return spent

    def drain(self):
        while self.pos < len(self.items):
            self._emit_one()
        # fire any stragglers
        for _, fn in self.deferred:
            fn()
        self.deferred = []


def build_nc(fix_waits=True, use_bias=False):
    import concourse.tile as tile
    from concourse import bacc, mybir
    from contextlib import ExitStack

    _enable_ldw_opt()

    BF = mybir.dt.bfloat16
    F32 = mybir.dt.float32
    EXP = mybir.ActivationFunctionType.Exp
    ESCALE = 0.125 / (W8SCALE * W8SCALE) if (USE_FP8_QK and not use_bias) \
        else 0.125

    nc = bacc.Bacc()
    # all inputs are row-paired on the host: channel chunks (2a, 2a+1) sit
    # side by side in one 128-partition tile, so every DMA is ONE ~256 KB
    # contiguous descriptor (dma_start issue costs ~650 ns of engine queue
    # time each -- descriptor count, not bytes, dominated the old startup)
    xt_d = nc.declare_dram_parameter("xt", [4 * 512, 1024], BF, isOutput=False)
    wq_d = nc.declare_dram_parameter("wq", [512, 1024], BF, isOutput=False)
    wk_d = nc.declare_dram_parameter("wk", [512, 1024], BF, isOutput=False)
    wv_d = nc.declare_dram_parameter("wv", [512, 1024], BF, isOutput=False)
    wp_d = nc.declare_dram_parameter("wp", [256, 2 * C], BF, isOutput=False)
    mk_d = nc.declare_dram_parameter("msk", [128, 4 * 512], BF, isOutput=False)
    fp8_qk = USE_FP8_QK and not use_bias
    if fp8_qk:
        F8 = mybir.dt.float8e4
        DR = mybir.MatmulPerfMode.DoubleRow
        x8_d = nc.declare_dram_parameter("x8", [1024, 2048], F8,
                                         isOutput=False)
        wq8_d = nc.declare_dram_parameter("wq8", [512, 1024], F8,
                                          isOutput=False)
        wk8_d = nc.declare_dram_parameter("wk8", [512, 1024], F8,
                                          isOutput=False)
    if use_bias:
        bq_d = nc.declare_dram_parameter("bq", [CH, 1], F32, isOutput=False)
        bk_d = nc.declare_dram_parameter("bk", [CH, 1], F32, isOutput=False)
        bv_d = nc.declare_dram_parameter("bv", [1, CH], F32, isOutput=False)
        bp_d = nc.declare_dram_parameter("bp", [1, C], F32, isOutput=False)
    out_d = nc.declare_dram_parameter("out", [T, C], BF, isOutput=True)

    with tile.TileContext(nc) as tc, ExitStack() as ctx:
        persist = ctx.enter_context(tc.tile_pool(name="persist", bufs=1))

        # persistent SBUF tensors
        qt = [persist.tile([128, T], BF, tag=f"qt{i}", name=f"qt{i}") for i in range(4)]
        kt = [persist.tile([128, T], BF, tag=f"kt{i}", name=f"kt{i}") for i in range(4)]
        vt = [persist.tile([128, VW], BF, tag=f"vt{i}", name=f"vt{i}") for i in range(16)]
        yt = [persist.tile([128, T], BF, tag=f"yt{i}", name=f"yt{i}") for i in range(4)]
        msk = persist.tile([128, 4 * 512], BF, tag="msk", name="msk")
        wup = persist.tile([128, 512], BF, tag="wup", name="wup")

        with tc.tile_pool(name="pS", bufs=2, space="PSUM") as pS, \
             tc.tile_pool(name="pO", bufs=4, space="PSUM") as pO, \
             tc.tile_pool(name="wqk", bufs=1) as wqkp, \
             tc.tile_pool(name="x8", bufs=4) as x8p, \
             tc.tile_pool(name="wv", bufs=1) as wvp, \
             tc.tile_pool(name="wp", bufs=1) as wpp, \
             tc.tile_pool(name="xt", bufs=8) as xtp, \
             tc.tile_pool(name="outst", bufs=8) as outp, \
             tc.tile_pool(name="exp", bufs=6) as expp, \
             tc.tile_pool(name="rn", bufs=4) as rnp:

            # ---- PE warm-up: zero matmuls while the first DMAs land ----
            nc.vector.memset(wup[:], 0.0)
            wps = pS.tile([128, 512], F32, tag="S", name="Swu")

            def wu_mm():
                nc.tensor.matmul(wps[:], wup[:, 0:128], wup[:],
                                 start=True, stop=True)

            # ---- DMAs: one 256 KB descriptor per tile-pair, issue spread
            # over the GpSimd (weights) and Sync (x, out) queues so the
            # ~650 ns/descriptor issue cost parallelizes.
            xts_all = {}

            def load_xts(tcx):
                xts_all[tcx] = []
                for a in range(4):
                    t = xtp.tile([128, 1024], BF, tag="xt", name="xt")
                    r0 = tcx * 512 + a * 128
                    nc.sync.dma_start(t[:], xt_d[r0:r0 + 128, :])
                    xts_all[tcx].append(t)

            wq_sb, wk_sb, wv_sb = [], [], []
            x8_all = {}

            def load_x8(tcx):
                x8_all[tcx] = []
                for sp in range(2):
                    t = x8p.tile([128, 2048], F8, tag="x8", name="x8")
                    r0 = tcx * 256 + sp * 128
                    nc.sync.dma_start(t[:], x8_d[r0:r0 + 128, :])
                    x8_all[tcx].append(t)

            if fp8_qk:
                wq8_sb, wk8_sb = [], []
                for colc in range(4):
                    t = wqkp.tile([128, 1024], F8, tag=f"wq8_{colc}",
                                  name=f"wq8_{colc}")
                    nc.gpsimd.dma_start(t[:],
                                        wq8_d[colc * 128:(colc + 1) * 128, :])
                    wq8_sb.append(t)
                    t = wqkp.tile([128, 1024], F8, tag=f"wk8_{colc}",
                                  name=f"wk8_{colc}")
                    nc.gpsimd.dma_start(t[:],
                                        wk8_d[colc * 128:(colc + 1) * 128, :])
                    wk8_sb.append(t)
                load_x8(0)
                load_xts(0)
                load_x8(1)
            else:
                for a in range(4):
                    t = wqkp.tile([128, 1024], BF, tag=f"wq{a}", name=f"wq{a}")
                    nc.gpsimd.dma_start(t[:], wq_d[a * 128:(a + 1) * 128, :])
                    wq_sb.append(t)
                load_xts(0)
                for a in range(4):
                    t = wqkp.tile([128, 1024], BF, tag=f"wk{a}", name=f"wk{a}")
                    nc.gpsimd.dma_start(t[:], wk_d[a * 128:(a + 1) * 128, :])
                    wk_sb.append(t)
            nc.gpsimd.dma_start(msk[:], mk_d[:, :])
            for a in range(4):
                t = wvp.tile([128, 1024], BF, tag=f"wv{a}", name=f"wv{a}")
                nc.gpsimd.dma_start(t[:], wv_d[a * 128:(a + 1) * 128, :])
                wv_sb.append(t)
            load_xts(1)
            wp_sb = []
            for a in range(2):
                t = wpp.tile([128, 2 * C], BF, tag=f"wp{a}", name=f"wp{a}")
                nc.sync.dma_start(t[:], wp_d[a * 128:(a + 1) * 128, :])
                wp_sb.append(t)

            # warm the PE HAM clock gate while the first DMAs land
            for _ in range(12):
                wu_mm()

            # ones columns of V (tight layout: col 64 of each 65-wide head)
            for i in range(16):
                v3 = vt[i][:].rearrange("p (h c) -> p h c", h=8, c=65)
                nc.vector.memset(v3[:, :, 64:65], 1.0)

            def wq_sl(colc, ck):
                a, par = ck >> 1, ck & 1
                cs = slice(par * 512 + colc * 128, par * 512 + (colc + 1) * 128)
                return wq_sb[a][:, cs]

            def wk_sl(colc, ck):
                a, par = ck >> 1, ck & 1
                cs = slice(par * 512 + colc * 128, par * 512 + (colc + 1) * 128)
                return wk_sb[a][:, cs]

            def x_sl(tcx, ck):
                a, par = ck >> 1, ck & 1
                return xts_all[tcx][a][:, par * 512:(par + 1) * 512]

            def xst_sl(tcx, ck, tt):
                a, par = ck >> 1, ck & 1
                cs = slice(par * 512 + tt * 128, par * 512 + (tt + 1) * 128)
                return xts_all[tcx][a][:, cs]

            def wv_sl(ck):
                a, par = ck >> 1, ck & 1
                return wv_sb[a][:, par * 512:(par + 1) * 512]

            def wp_sl(ck, cc):
                a, par = ck >> 1, ck & 1
                cs = slice(par * C + cc * 512, par * C + (cc + 1) * 512)
                return wp_sb[a][:, cs]

            if fp8_qk:
                def qk_mm(ps, which, colc, tcx, a, start, stop):
                    w8 = (wq8_sb if which == "q" else wk8_sb)[colc]
                    lhsT = w8[:, a * 256:(a + 1) * 256].rearrange(
                        "p (i m) -> p i m", i=2, m=128)
                    sp, al = a >> 1, a & 1
                    rhs = x8_all[tcx][sp][:, al * 1024:(al + 1) * 1024
                                          ].rearrange("p (i n) -> p i n",
                                                      i=2, n=512)
                    nc.tensor.matmul(ps, lhsT, rhs, start=start, stop=stop,
                                     perf_mode=DR)
                N_QK = 4
            else:
                def qk_mm(ps, which, colc, tcx, ck, start, stop):
                    w_sl = wq_sl if which == "q" else wk_sl
                    nc.tensor.matmul(ps, w_sl(colc, ck), x_sl(tcx, ck),
                                     start=start, stop=stop)
                N_QK = 8

            if use_bias:
                bq_sb = persist.tile([128, 4], F32, tag="bq", name="bq")
                bk_sb = persist.tile([128, 4], F32, tag="bk", name="bk")
                bv_row = persist.tile([1, CH], F32, tag="bvr", name="bvr")
                bp_row = persist.tile([1, C], F32, tag="bpr", name="bpr")
                bvb = persist.tile([128, CH], F32, tag="bvb", name="bvb")
                bpb = persist.tile([128, C], F32, tag="bpb", name="bpb")
                for colc in range(4):
                    nc.sync.dma_start(bq_sb[:, colc:colc + 1],
                                      bq_d[colc * 128:(colc + 1) * 128, :])
                    nc.sync.dma_start(bk_sb[:, colc:colc + 1],
                                      bk_d[colc * 128:(colc + 1) * 128, :])
                nc.sync.dma_start(bv_row[:], bv_d[:, :])
                nc.sync.dma_start(bp_row[:], bp_d[:, :])
                nc.gpsimd.partition_broadcast(bvb[:], bv_row[:], channels=128)
                nc.gpsimd.partition_broadcast(bpb[:], bp_row[:], channels=128)

            # ---- filler thunk builders (single-matmul granularity) ----
            MM = 221.0   # ns, one N=512 bf16 matmul issue-to-issue

            def palloc(nm):
                # Every pO allocation may reuse the ring slot of a chunk
                # accumulator whose deferred yt-multiplies haven't been
                # emitted yet -- flush them first so Tile sees the reads.
                fill.flush_deferred()
                return pO.tile([128, 512], F32, tag="O", name=nm)

            def qk_group(which, colc, tcx):
                """matmuls + copy producing qt/kt[colc][:, tcx-chunk]."""
                dst = qt if which == "q" else kt
                ts = slice(tcx * 512, (tcx + 1) * 512)
                cell = {}

                def mk(ck):
                    def f():
                        if ck == 0:
                            cell["ps"] = palloc("Sg")
                        qk_mm(cell["ps"][:], which, colc, tcx, ck,
                              ck == 0, ck == N_QK - 1)
                        if ck == N_QK - 1:
                            if use_bias:
                                bcol = bq_sb if which == "q" else bk_sb
                                nc.vector.tensor_scalar_add(
                                    dst[colc][:, ts], cell["ps"][:],
                                    bcol[:, colc:colc + 1])
                            else:
                                nc.vector.tensor_copy(dst[colc][:, ts],
                                                      cell["ps"][:])
                    return f
                return [(MM, mk(ck)) for ck in range(N_QK)]

            def v_group(tcx, tt):
                """8 matmuls + copy producing vt[tcx*4+tt]."""
                vti = vt[tcx * 4 + tt]
                cell = {}

                def mk(ck):
                    def f():
                        if ck == 0:
                            cell["ps"] = palloc("Vg")
                        nc.tensor.matmul(cell["ps"][:],
                                         xst_sl(tcx, ck, tt),
                                         wv_sl(ck),
                                         start=(ck == 0), stop=(ck == 7))
                        if ck == 7:
                            dst = vti[:].rearrange(
                                "p (h c) -> p h c", h=8, c=65)[:, :, 0:64]
                            src = cell["ps"][:].rearrange(
                                "p (h c) -> p h c", h=8, c=64)
                            if use_bias:
                                bsrc = bvb[:].rearrange(
                                    "p (h c) -> p h c", h=8, c=64)
                                nc.vector.tensor_add(dst, src, bsrc)
                            else:
                                nc.vector.tensor_copy(dst, src)
                    return f
                return [(MM, mk(ck)) for ck in range(8)]

            stage3 = {}  # t2 -> [128, 1024] f32 staged ck0-2 partial

            def proj3_partial(t2, cc):
                """ck=0..2 partial contraction for an i-chunk-3 output
                tile, staged to SBUF f32 (frees its PSUM slot right away);
                woven into the last attention chunk."""
                t2s = slice(t2 * 128, (t2 + 1) * 128)
                ccs = slice(cc * 512, (cc + 1) * 512)
                cell = {}

                def mk(ck):
                    def f():
                        if t2 not in stage3:
                            stage3[t2] = outp.tile([128, 1024], F32,
                                                   tag="st3", name="st3",
                                                   bufs=4)
                        if ck == 0:
                            cell["ps"] = palloc("Pp")
                        nc.tensor.matmul(cell["ps"][:], yt[ck][:, t2s],
                                         wp_sl(ck, cc),
                                         start=(ck == 0), stop=(ck == 2))
                        if ck == 2:
                            if use_bias:
                                nc.vector.tensor_add(stage3[t2][:, ccs],
                                                     cell["ps"][:],
                                                     bpb[:, ccs])
                            else:
                                nc.vector.tensor_copy(stage3[t2][:, ccs],
                                                      cell["ps"][:])
                    return f
                return [(MM, mk(ck)) for ck in range(3)]

            def proj_t2(t2):
                """8 matmuls (two 512-col halves), staged into one SBUF
                tile, one contiguous 256 KB output DMA."""
                t2s = slice(t2 * 128, (t2 + 1) * 128)
                cell = {}

                def mk(cc, ck):
                    ccs = slice(cc * 512, (cc + 1) * 512)

                    def f():
                        if cc == 0 and ck == 0:
                            cell["st"] = outp.tile([128, 1024], BF, tag="ost",
                                                   name="ost")
                        if ck == 0:
                            cell["ps"] = palloc("Sp")
                        nc.tensor.matmul(
                            cell["ps"][:], yt[ck][:, t2s], wp_sl(ck, cc),
                            start=(ck == 0), stop=(ck == 3))
                        if ck == 3:
                            if use_bias:
                                nc.vector.tensor_add(cell["st"][:, ccs],
                                                     cell["ps"][:], bpb[:, ccs])
                            else:
                                nc.vector.tensor_copy(cell["st"][:, ccs],
                                                      cell["ps"][:])
                            if cc == 1:
                                nc.sync.dma_start(out_d[t2s, :], cell["st"][:])
                    return f
                return [(MM, mk(cc, ck)) for cc in range(2) for ck in range(4)]

            # ---- build the filler queue in dependency-safe order ----
            fill = _Filler()
            for tt in range(4):
                fill.add_group(("v", 0, tt), v_group(0, tt))
            for colc in range(1, 4):
                fill.add_group(("k", colc, 0), qk_group("k", colc, 0))
                fill.add_group(("q", colc, 0), qk_group("q", colc, 0))
            for tcx in range(1, 4):
                def xload(tcx=tcx):
                    if tcx + 1 <= 3:
                        load_xts(tcx + 1)
                        if fp8_qk:
                            load_x8(tcx + 1)
                fill.add_group(("x", tcx), [(0.0, xload)])
                fill.add_group(("q", 0, tcx), qk_group("q", 0, tcx))
                fill.add_group(("k", 0, tcx), qk_group("k", 0, tcx))
                for tt in range(4):
                    fill.add_group(("v", tcx, tt), v_group(tcx, tt))
                for colc in range(1, 4):
                    fill.add_group(("q", colc, tcx), qk_group("q", colc, tcx))
                    fill.add_group(("k", colc, tcx), qk_group("k", colc, tcx))


            # ---- startup: the two gen groups the first chunk needs ----
            ps_q = palloc("q00")
            for ck in range(N_QK):
                qk_mm(ps_q[:], "q", 0, 0, ck, ck == 0, ck == N_QK - 1)
            if use_bias:
                nc.vector.tensor_scalar_add(qt[0][:, 0:512], ps_q[:],
                                            bq_sb[:, 0:1])
            else:
                nc.vector.tensor_copy(qt[0][:, 0:512], ps_q[:])
            ps_k = palloc("k00")
            for ck in range(N_QK):
                qk_mm(ps_k[:], "k", 0, 0, ck, ck == 0, ck == N_QK - 1)
            if use_bias:
                nc.vector.tensor_scalar_add(kt[0][:, 0:512], ps_k[:],
                                            bk_sb[:, 0:1])
            else:
                nc.vector.tensor_copy(kt[0][:, 0:512], ps_k[:])

            # ---- the flat attention block stream ----
            def normalize(hp, pic, opsA, opsB):
                """Phase 1 (copies/recips/broadcasts) runs now; the two DVE
                multiplies are deferred so they don't block the DVE queue
                while the GpSimd broadcast runs."""
                r = {}
                r["dnA"] = rnp.tile([1, 512], F32, tag="dn", name="dnA")
                nc.vector.tensor_copy(r["dnA"][:], opsA[D:D + 1, :])
                r["rfA"] = rnp.tile([1, 512], F32, tag="rf", name="rfA")
                nc.vector.reciprocal_approx_fast(r["rfA"][:], r["dnA"][:])
                r["rsA"] = rnp.tile([D, 512], F32, tag="Rs", name="RsA")
                nc.gpsimd.partition_broadcast(r["rsA"][:], r["rfA"][:],
                                              channels=D)
                r["dnB"] = rnp.tile([1, 512], F32, tag="dn", name="dnB")
                nc.vector.tensor_copy(r["dnB"][:], opsB[D:D + 1, :])
                r["rfB"] = rnp.tile([1, 512], F32, tag="rf", name="rfB")
                nc.vector.reciprocal_approx_fast(r["rfB"][:], r["dnB"][:])
                r["rsB"] = rnp.tile([D, 512], F32, tag="Rs", name="RsB")
                nc.gpsimd.partition_broadcast(r["rsB"][:], r["rfB"][:],
                                              channels=D)
                isl = slice(pic * 512, (pic + 1) * 512)

                def ph2(hp=hp, isl=isl, r=r, opsA=opsA, opsB=opsB):
                    nc.vector.tensor_mul(yt[hp][0:D, isl], opsA[0:D, :],
                                         r["rsA"][:])
                    nc.vector.tensor_mul(yt[hp][D:128, isl], opsB[0:D, :],
                                         r["rsB"][:])
                fill.defer(ph2, 4)

            blocks = []
            for ic in range(4):
                for hp in range(4):
                    jmax = 4 * (ic + 1)
                    for jt in range(jmax):
                        blocks.append((hp, ic, jt, jt == 0, jt == jmax - 1))

            # ---- global filler plan: late (ACT-bound) chunks get exactly
            # their ACT-over-PE deficit; the excess filler is front-loaded
            # into the early PE-bound chunks where emission order is free.
            def blk_costs(ic, jt):
                m = jt - 4 * ic
                c0 = 128 * m if m > 0 else 0
                a = (2 * (512 - c0) + 352) / 1.2
                p = (512 - c0) / 2.4 + 95.0 + 2 * (512 - c0) / 2.4 + 115.0
                return a, p

            # queue items now + proj(0..2) groups appended during the stream
            F_total = sum(c for _, c, _ in fill.items) + 12 * 8 * MM
            deficit = {}
            for ic in range(4):
                for hp in range(4):
                    a = p = 0.0
                    for jt in range(4 * (ic + 1)):
                        da, dp = blk_costs(ic, jt)
                        a += da
                        p += dp
                    deficit[(hp, ic)] = max(0.0, a - p - 300.0)
            extra = max(0.0, F_total - sum(deficit.values()))
            quota = dict(deficit)
            for ic in range(4):
                for hp in range(4):
                    if extra <= 0:
                        break
                    cap = 7000.0 if ic < 2 else 3000.0
                    add = min(extra, cap)
                    quota[(hp, ic)] += add
                    extra -= add
            # cumulative planned filler line per block index
            plan = []
            cum = 0.0
            for (hp, ic, jt, first, last) in blocks:
                cum += quota[(hp, ic)] / (4 * (ic + 1))
                plan.append(cum)

            pe_t = 0.0       # modeled PE completion time of emitted work
            act_t = 0.0      # modeled ACT completion time
            pend = None      # (hp, ic, jt, ex, c0, last, opsA, opsB, exp_end)
            cur_ops = None   # (opsA, opsB) of the chunk whose AVs are pending
            GUARD = 150.0
            bi = -1

            for (hp, ic, jt, first, last) in blocks:
                bi += 1
                # --- force-drain dependencies for this block's S pair ---
                tcx_j = jt // 4
                if first and (hp, ic) != (0, 0):
                    fill.need(("q", hp, ic))
                if (hp, tcx_j) != (0, 0):
                    fill.need(("k", hp, tcx_j))

                # --- emit S pair for this block ---
                jsl = slice(jt * 128, (jt + 1) * 128)
                m = jt - 4 * ic
                c0 = 128 * m if m > 0 else 0
                iv = slice(ic * 512 + c0, (ic + 1) * 512)
                sps = pS.tile([128, 1024], F32, tag="S", name="S")
                nc.tensor.matmul(sps[:, c0:512], kt[hp][0:D, jsl],
                                 qt[hp][0:D, iv], start=True, stop=True)
                nc.tensor.matmul(sps[:, 512 + c0:1024], kt[hp][D:128, jsl],
                                 qt[hp][D:128, iv], start=True, stop=True)
                s_cost = (512 - c0) / 2.4 + 95.0
                pe_t += s_cost

                # --- emit exp (+ mask for diagonal blocks) ---
                ex = expp.tile([128, 1024], BF, tag="ex", name="ex")
                ex3 = ex[:].rearrange("p (t c) -> p t c", t=2, c=512)
                sps3 = sps[:].rearrange("p (t c) -> p t c", t=2, c=512)
                if m < 0:
                    nc.scalar.activation(ex[:], sps[:], EXP, scale=ESCALE)
                else:
                    ms3 = msk[:, m * 512 + c0:m * 512 + c0 + 128
                              ].unsqueeze(1).broadcast_to([128, 2, 128])
                    nc.scalar.activation(ex3[:, :, c0:512],
                                         sps3[:, :, c0:512],
                                         EXP, scale=ESCALE)
                    nc.vector.tensor_mul(ex3[:, :, c0:c0 + 128],
                                         ex3[:, :, c0:c0 + 128], ms3)
                exp_start = max(act_t, pe_t + 60.0)
                act_t = exp_start + (2 * (512 - c0) + 352) / 1.2
                my_exp_end = act_t

                # --- pace filler until the pending AV's exp is done ---
                if pend is not None:
                    target = pend[8] + GUARD
                    if pe_t < target:
                        pe_t += fill.emit(target - pe_t)

                # --- emit the pending block's AV pair ---
                if pend is not None:
                    phs, pic_, pj, pex, pc0, plast, popsA, popsB, pexp = pend
                    v0 = 130 * phs
                    pe_t = max(pe_t, pexp + 80.0)
                    nc.tensor.matmul(popsA[0:65, pc0:512],
                                     vt[pj][:, v0:v0 + 65],
                                     pex[:, pc0:512],
                                     start=(pj == 0), stop=plast)
                    nc.tensor.matmul(popsB[0:65, pc0:512],
                                     vt[pj][:, v0 + 65:v0 + 130],
                                     pex[:, 512 + pc0:1024],
                                     start=(pj == 0), stop=plast)
                    pe_t += 2 * (512 - pc0) / 2.4 + 115.0
                    if plast:
                        normalize(phs, pic_, popsA, popsB)
                        if phs == 3 and pic_ < 3:
                            # all of i-chunk pic_ is normalized: its
                            # projection becomes available filler
                            for t2 in range(4 * pic_, 4 * pic_ + 4):
                                fill.add_group(("p", pic_, t2), proj_t2(t2))


                # --- planned-quota weaving (front-loads excess filler) ---
                if fill.spent_total < plan[bi]:
                    pe_t += fill.emit(plan[bi] - fill.spent_total)

                # --- queue this block as pending (alloc ops at jt==0) ---
                if first:
                    # deferred yt-multiplies of older chunks must be emitted
                    # before their ops PSUM slots can be reallocated
                    fill.flush_deferred()
                    opsA = pO.tile([128, 512], F32, tag="O", name="OA")
                    opsB = pO.tile([128, 512], F32, tag="O", name="OB")
                    cur_ops = (opsA, opsB)
                fill.need(("v", tcx_j, jt % 4))
                pend = (hp, ic, jt, ex, c0, last, cur_ops[0], cur_ops[1],
                        my_exp_end)

            # flush the final AV pair
            phs, pic_, pj, pex, pc0, plast, popsA, popsB, pexp = pend
            v0 = 130 * phs
            nc.tensor.matmul(popsA[0:65, pc0:512], vt[pj][:, v0:v0 + 65],
                             pex[:, pc0:512], start=(pj == 0), stop=True)
            nc.tensor.matmul(popsB[0:65, pc0:512],
                             vt[pj][:, v0 + 65:v0 + 130],
                             pex[:, 512 + pc0:1024],
                             start=(pj == 0), stop=True)
            normalize(phs, pic_, popsA, popsB)
            fill.drain()

            # ---- tail: projection of i-chunk 3 ----
            for t2 in range(12, 16):
                for _, fn in proj_t2(t2):
                    fn()

    nc.finalize()  # Bacc.compile(): ISA-subclass codegen, gpsimd library
    # loads, act-table loads, nop fusion -- must precede the wait splitting
    if fix_waits:
        _fix_multi_waits(nc)
    return nc


def _host_inputs(x, W_qkv, b_qkv, W_proj, b_proj, use_bias):
    x = np.asarray(x, np.float32)
    W_qkv = np.asarray(W_qkv, np.float32)
    b_qkv = np.asarray(b_qkv, np.float32)
    W_proj = np.asarray(W_proj, np.float32)
    b_proj = np.asarray(b_proj, np.float32)

    # causal masks for the 4 diagonal-overlap offsets: ST block [j 128, i 512]
    # at j0 - i0 = 128*m keeps (ii >= jj + 128*m)
    jj = np.arange(128)[:, None]
    ii = np.arange(512)[None, :]
    msk = np.concatenate(
        [(ii >= jj + 128 * m).astype(np.float32) for m in range(4)], axis=1)

    def pair_rows(M):
        # [R, C] -> [R/2, 2C]: row chunks (2a, 2a+1) side by side, so one
        # 128-partition SBUF tile loads as a single contiguous DMA
        R, Cc = M.shape
        return M.reshape(R // 256, 2, 128, Cc).transpose(0, 2, 1, 3).reshape(
            R // 2, 2 * Cc)

    in_maps = []
    for core in range(NCORES):
        b, hg = core >> 1, core & 1
        q0 = hg * CH
        xT = x[b].T  # [C, T]
        # t-chunk-major, row-paired
        xt = np.concatenate(
            [pair_rows(xT[:, tc * 512:(tc + 1) * 512]) for tc in range(4)],
            axis=0).astype(_BF16)
        wq = pair_rows(W_qkv[:, q0:q0 + CH]).astype(_BF16)
        wk = pair_rows(W_qkv[:, C + q0:C + q0 + CH]).astype(_BF16)
        wv = pair_rows(W_qkv[:, 2 * C + q0:2 * C + q0 + CH]).astype(_BF16)
        wp = pair_rows(W_proj[q0:q0 + CH, :]).astype(_BF16)
        im = {"xt": xt, "wq": wq, "wk": wk, "wv": wv, "wp": wp,
              "msk": msk.astype(_BF16)}
        if USE_FP8_QK and not use_bias:
            # x8[tcx, sp, p, a_loc, i, n] = x[256(2sp+a_loc)+128i+p, tcx*512+n]
            x8 = xT.reshape(2, 2, 2, 128, 4, 512).transpose(
                4, 0, 3, 1, 2, 5).reshape(1024, 2048)
            im["x8"] = x8.astype(_F8)

            def w8pack(w):
                # w8[colc][p, a*256+i*128+m] = W8SCALE * w[a*256+i*128+p,
                #                                          colc*128+m]
                wr = (W8SCALE * w).reshape(4, 2, 128, 4, 128)
                return wr.transpose(3, 2, 0, 1, 4).reshape(512, 1024).astype(
                    _F8)
            im["wq8"] = w8pack(W_qkv[:, q0:q0 + CH])
            im["wk8"] = w8pack(W_qkv[:, C + q0:C + q0 + CH])
        if use_bias:
            im["bq"] = b_qkv[q0:q0 + CH].reshape(CH, 1).astype(np.float32)
            im["bk"] = b_qkv[C + q0:C + q0 + CH].reshape(CH, 1).astype(
                np.float32)
            im["bv"] = b_qkv[2 * C + q0:2 * C + q0 + CH].reshape(1, CH).astype(
                np.float32)
            im["bp"] = (0.5 * b_proj).reshape(1, C).astype(np.float32)
        in_maps.append(im)
    return in_maps


def kernel(x, W_qkv, b_qkv, W_proj, b_proj):
    from concourse.bass_utils import run_bass_kernel_spmd

    use_bias = bool(np.any(np.asarray(b_qkv)) or np.any(np.asarray(b_proj)))
    if use_bias not in _nc_cache:
        _nc_cache[use_bias] = build_nc(use_bias=use_bias)
    nc = _nc_cache[use_bias]

    in_maps = _host_inputs(x, W_qkv, b_qkv, W_proj, b_proj, use_bias)
    res = run_bass_kernel_spmd(nc, in_maps, core_ids=list(range(NCORES)))
    LAST_RESULT[0] = res

    out = np.empty((B, T, C), np.float32)
    for b in range(B):
        out[b] = (res.results[2 * b]["out"].astype(np.float32)
                  + res.results[2 * b + 1]["out"].astype(np.float32))
    return out
